# revision 8
# baseline (speedup 1.0000x reference)
"""Causal cross-attention kernel for 8 trn2 NeuronCores.

Sharding: 4-way data-parallel over batch x 2-way tensor-parallel over heads
(8 heads per core).  Each core computes, for its (batch, head-group):
  Q^T = WqT.T-less trick:  Q^T[o,t] = sum_d WqT[d,o] * XqT[d,t]   (o = 512 head dims)
  K^T likewise, V[t,o] natural layout, all via PE matmuls in float32r.
Attention is computed in transposed layout: S^T[k,q] = K^T.T-contraction so the
softmax denominator comes from a ones-column appended to V (no P transpose, no
max-subtraction -- scores are ~N(0,1) so exp never overflows in fp32).
Output projection consumes O^T directly as lhsT; each core emits a full-width
partial y for its batch and the host sums the two head-group partials.

All host-side work (transposes, slicing, pair-sums) is data marshaling; the
device kernel is a single NEFF launch per core.
"""

import sys

sys.path.insert(0, "/opt/trn_rl_repo")

import numpy as np

import concourse.bass as bass
import concourse.tile as tile
from concourse import bacc, mybir
from concourse.bass import ts
from concourse.masks import make_upper_triangular

F32 = mybir.dt.float32
F32R = mybir.dt.float32r
P = 128

# full-problem constants
B_FULL = 4
S_FULL = 2048
D_FULL = 1024
HG_FULL = 8  # heads per core (16 heads / 2-way TP)
N_CORES = 8


def build_bass(S=S_FULL, D=D_FULL, HG=HG_FULL):
    """One-core program; SPMD across 8 cores with different data."""
    GO = HG * 64  # output-feature width of this core's head group
    ND = D // P  # d-blocks (contraction)
    NM = GO // P  # o-tiles of Q/K projections
    NQT = S // 512  # q-tiles (512 wide)
    NTB = S // P  # token blocks of 128
    TCH = 256  # projection t-chunk
    NCH = S // TCH

    nc = bacc.Bacc("TRN2", target_bir_lowering=False, debug=False)
    xqT = nc.dram_tensor("xqT", [D, S], F32R, kind="ExternalInput")
    xkvT = nc.dram_tensor("xkvT", [D, S], F32R, kind="ExternalInput")
    wqT = nc.dram_tensor("wqT", [D, GO], F32R, kind="ExternalInput")
    wkT = nc.dram_tensor("wkT", [D, GO], F32R, kind="ExternalInput")
    wvT = nc.dram_tensor("wvT", [D, GO], F32R, kind="ExternalInput")
    woT = nc.dram_tensor("woT", [GO, D], F32R, kind="ExternalInput")
    y = nc.dram_tensor("y", [S, D], F32, kind="ExternalOutput")

    Exp = mybir.ActivationFunctionType.Exp

    with tile.TileContext(nc) as tc:
        from contextlib import ExitStack

        with ExitStack() as ctx:
            ctx.enter_context(
                nc.allow_low_precision(reason="fp32r matmul input rounding")
            )
            # ---- persistent SBUF buffers ----
            pers = ctx.enter_context(tc.tile_pool(name="pers", bufs=1))
            qT = [pers.tile([P, S], F32R, tag=f"qT{i}", name=f"qT{i}") for i in range(NM)]
            kT = [pers.tile([P, S], F32R, tag=f"kT{i}", name=f"kT{i}") for i in range(NM)]
            vaug = [pers.tile([P, HG * 65], F32R, tag=f"va{i}", name=f"va{i}") for i in range(NTB)]
            oT = [pers.tile([P, S], F32R, tag=f"oT{i}", name=f"oT{i}") for i in range(NM)]
            consts = ctx.enter_context(tc.tile_pool(name="consts", bufs=1))
            tri_f = consts.tile([P, P], F32)  # tri[k,q] = 1 if q >= k else 0
            make_upper_triangular(nc, tri_f[:], val=1.0, diag=True)
            tri = consts.tile([P, P], F32R)
            nc.vector.tensor_copy(tri[:], tri_f[:])
            ones_f = consts.tile([1, 64], F32)
            nc.vector.memset(ones_f[:], 1.0)
            ones1 = consts.tile([1, 64], F32R)
            nc.vector.tensor_copy(ones1[:], ones_f[:])
            vone = consts.tile([P, HG * 65], F32)
            nc.vector.memset(vone[:], 1.0)
            for i in range(NTB):
                # ones columns survive the V evictions (cols h*65+64)
                nc.vector.tensor_copy(vaug[i][:], vone[:])

            # ---- phase 1a: Q projection ----
            with (
                tc.tile_pool(name="wq", bufs=1) as wq_pool,
                tc.tile_pool(name="xq", bufs=2) as xq_pool,
                tc.tile_pool(name="pj", bufs=2, space="PSUM") as pj_pool,
            ):
                wq_t = [wq_pool.tile([P, GO], F32R, tag=f"w{d}", name=f"wq{d}") for d in range(ND)]
                for d in range(ND):
                    nc.sync.dma_start(wq_t[d][:], wqT[ts(d, P), :])
                for tc_i in range(NCH):
                    xq = [xq_pool.tile([P, TCH], F32R, tag=f"x{d}", name=f"xq{d}_{tc_i}") for d in range(ND)]
                    for d in range(ND):
                        nc.sync.dma_start(xq[d][:], xqT[ts(d, P), ts(tc_i, TCH)])
                    for m in range(NM):
                        ps = pj_pool.tile([P, TCH], F32, tag="pj")
                        for d in range(ND):
                            nc.tensor.matmul(
                                ps[:],
                                wq_t[d][:, ts(m, P)],
                                xq[d][:],
                                start=(d == 0),
                                stop=(d == ND - 1),
                            )
                        nc.vector.tensor_copy(qT[m][:, ts(tc_i, TCH)], ps[:])

            # ---- phase 1b: K and V projections ----
            with (
                tc.tile_pool(name="wkv", bufs=1) as wkv_pool,
                tc.tile_pool(name="xkv", bufs=2) as xkv_pool,
                tc.tile_pool(name="pj2", bufs=2, space="PSUM") as pj2_pool,
            ):
                wk_t = [wkv_pool.tile([P, GO], F32R, tag=f"wk{d}", name=f"wk{d}") for d in range(ND)]
                wv_t = [wkv_pool.tile([P, GO], F32R, tag=f"wv{d}", name=f"wv{d}") for d in range(ND)]
                for d in range(ND):
                    nc.sync.dma_start(wk_t[d][:], wkT[ts(d, P), :])
                    nc.sync.dma_start(wv_t[d][:], wvT[ts(d, P), :])
                for tc_i in range(NCH):
                    xkv = [
                        xkv_pool.tile([P, TCH], F32R, tag=f"x{d}", name=f"xkv{d}_{tc_i}") for d in range(ND)
                    ]
                    for d in range(ND):
                        nc.sync.dma_start(xkv[d][:], xkvT[ts(d, P), ts(tc_i, TCH)])
                    for m in range(NM):
                        ps = pj2_pool.tile([P, TCH], F32, tag="pj")
                        for d in range(ND):
                            nc.tensor.matmul(
                                ps[:],
                                wk_t[d][:, ts(m, P)],
                                xkv[d][:],
                                start=(d == 0),
                                stop=(d == ND - 1),
                            )
                        nc.vector.tensor_copy(kT[m][:, ts(tc_i, TCH)], ps[:])
                    for mt in range(TCH // P):
                        ps = pj2_pool.tile([P, GO], F32, tag="pjv")
                        for d in range(ND):
                            nc.tensor.matmul(
                                ps[:],
                                xkv[d][:, ts(mt, P)],
                                wv_t[d][:],
                                start=(d == 0),
                                stop=(d == ND - 1),
                            )
                        vt = vaug[tc_i * (TCH // P) + mt]
                        for h in range(HG):
                            nc.vector.tensor_copy(
                                vt[:, h * 65 : h * 65 + 64], ps[:, ts(h, 64)]
                            )

            # ---- phase 2: attention (transposed layout) ----
            with (
                tc.tile_pool(name="att", bufs=4) as apool,
                tc.tile_pool(name="attn2", bufs=2) as apool2,
                tc.tile_pool(name="ps_s", bufs=2, space="PSUM") as spool,
                tc.tile_pool(name="ps_o", bufs=2, space="PSUM") as opool,
                tc.tile_pool(name="ps_b", bufs=2, space="PSUM") as bpool,
            ):
                for h in range(HG):
                    ti, po = h // 2, (h % 2) * 64
                    for qt in range(NQT):
                        pso = opool.tile([P, 512], F32, tag="o")
                        nkb = 4 * qt + 4
                        for kb in range(nkb):
                            # diagonal blocks: only q-cols >= j*128 are live
                            j = kb - 4 * qt
                            c0 = max(j, 0) * P
                            pss = spool.tile([P, 512], F32, tag="s")
                            nc.tensor.matmul(
                                pss[:, c0:],
                                kT[ti][po : po + 64, ts(kb, P)],
                                qT[ti][po : po + 64, qt * 512 + c0 : (qt + 1) * 512],
                                start=True,
                                stop=True,
                            )
                            pexp = apool.tile([P, 512], F32R, tag="p")
                            nc.scalar.activation(
                                pexp[:, c0:], pss[:, c0:], Exp, scale=0.125
                            )
                            if j >= 0:
                                nc.vector.tensor_mul(
                                    pexp[:, ts(j, P)], pexp[:, ts(j, P)], tri[:]
                                )
                            nc.tensor.matmul(
                                pso[:65, c0:],
                                vaug[kb][:, h * 65 : h * 65 + 65],
                                pexp[:, c0:],
                                start=(kb == 0),
                                stop=(kb == nkb - 1),
                            )
                        # normalize: rows 0..63 divided by row 64 (the exp-sum)
                        rec = apool2.tile([1, 512], F32R, tag="rec")
                        nc.vector.reciprocal(rec[:], pso[64:65, :])
                        bc_ps = bpool.tile([64, 512], F32, tag="bc")
                        nc.tensor.matmul(
                            bc_ps[:],
                            ones1[:],
                            rec[:],
                            start=True,
                            stop=True,
                        )
                        bc_sb = apool2.tile([64, 512], F32, tag="bcs")
                        nc.vector.tensor_copy(bc_sb[:], bc_ps[:])
                        row = h * 64
                        nc.vector.tensor_mul(
                            oT[row // P][row % P : row % P + 64, ts(qt, 512)],
                            pso[0:64, :],
                            bc_sb[:],
                        )

            # ---- phase 3: output projection (partial over this head group) ----
            with (
                tc.tile_pool(name="wo", bufs=1) as wo_pool,
                tc.tile_pool(name="yev", bufs=3) as y_pool,
                tc.tile_pool(name="ps_y", bufs=2, space="PSUM") as ypool,
            ):
                wo_t = [wo_pool.tile([P, D], F32R, tag=f"wo{i}", name=f"wo{i}") for i in range(NM)]
                for i in range(NM):
                    nc.sync.dma_start(wo_t[i][:], woT[ts(i, P), :])
                for mt in range(NTB):
                    for nt in range(D // 512):
                        ps = ypool.tile([P, 512], F32, tag="y")
                        for ob in range(NM):
                            nc.tensor.matmul(
                                ps[:],
                                oT[ob][:, ts(mt, P)],
                                wo_t[ob][:, ts(nt, 512)],
                                start=(ob == 0),
                                stop=(ob == NM - 1),
                            )
                        ysb = y_pool.tile([P, 512], F32, tag="ysb")
                        nc.vector.tensor_copy(ysb[:], ps[:])
                        nc.sync.dma_start(y[ts(mt, P), ts(nt, 512)], ysb[:])
    nc.finalize()
    return nc


_NC_CACHE = {}


def _get_nc():
    if "full" not in _NC_CACHE:
        _NC_CACHE["full"] = build_bass()
    return _NC_CACHE["full"]


def make_in_maps(query, key_value, Wq, Wk, Wv, Wo):
    query = np.asarray(query, dtype=np.float32)
    key_value = np.asarray(key_value, dtype=np.float32)
    Wq, Wk, Wv, Wo = (np.asarray(w, dtype=np.float32) for w in (Wq, Wk, Wv, Wo))
    GO = Wq.shape[0] // 2
    in_maps = []
    for c in range(N_CORES):
        b, g = c // 2, c % 2
        sl = slice(g * GO, (g + 1) * GO)
        in_maps.append(
            {
                "xqT": np.ascontiguousarray(query[b].T),
                "xkvT": np.ascontiguousarray(key_value[b].T),
                "wqT": np.ascontiguousarray(Wq[sl, :].T),
                "wkT": np.ascontiguousarray(Wk[sl, :].T),
                "wvT": np.ascontiguousarray(Wv[sl, :].T),
                "woT": np.ascontiguousarray(Wo[:, sl].T),
            }
        )
    return in_maps


def kernel(query, key_value, Wq, Wk, Wv, Wo):
    from concourse import bass_utils

    nc = _get_nc()
    in_maps = make_in_maps(query, key_value, Wq, Wk, Wv, Wo)
    res = bass_utils.run_bass_kernel_spmd(nc, in_maps, core_ids=list(range(N_CORES)))
    ys = [r["y"] for r in res.results]
    out = np.stack([ys[2 * b] + ys[2 * b + 1] for b in range(B_FULL)])
    return out.astype(np.float32)


# revision 10
# speedup vs baseline: 16.7617x; 16.7617x over previous
"""Causal cross-attention kernel for 8 trn2 NeuronCores.

Sharding: 4-way data-parallel over batch x 2-way tensor-parallel over heads
(8 heads per core).  Each core computes, for its (batch, head-group):
  Q^T = WqT.T-less trick:  Q^T[o,t] = sum_d WqT[d,o] * XqT[d,t]   (o = 512 head dims)
  K^T likewise, V[t,o] natural layout, all via PE matmuls in float32r.
Attention is computed in transposed layout: S^T[k,q] = K^T.T-contraction so the
softmax denominator comes from a ones-column appended to V (no P transpose, no
max-subtraction -- scores are ~N(0,1) so exp never overflows in fp32).
Output projection consumes O^T directly as lhsT; each core emits a full-width
partial y for its batch and the host sums the two head-group partials.

All host-side work (transposes, slicing, pair-sums) is data marshaling; the
device kernel is a single NEFF launch per core.
"""

import sys

sys.path.insert(0, "/opt/trn_rl_repo")

import numpy as np

import concourse.bass as bass
import concourse.tile as tile
from concourse import bacc, mybir
from concourse.bass import ts
from concourse.masks import make_upper_triangular

F32 = mybir.dt.float32
F32R = mybir.dt.float32r
P = 128

# full-problem constants
B_FULL = 4
S_FULL = 2048
D_FULL = 1024
HG_FULL = 8  # heads per core (16 heads / 2-way TP)
N_CORES = 8


def build_bass(S=S_FULL, D=D_FULL, HG=HG_FULL):
    """One-core program; SPMD across 8 cores with different data."""
    GO = HG * 64  # output-feature width of this core's head group
    ND = D // P  # d-blocks (contraction)
    NM = GO // P  # o-tiles of Q/K projections
    NQT = S // 512  # q-tiles (512 wide)
    NTB = S // P  # token blocks of 128
    TCH = 256  # projection t-chunk
    NCH = S // TCH

    nc = bacc.Bacc("TRN2", target_bir_lowering=False, debug=False)
    xqT = nc.dram_tensor("xqT", [D, S], F32R, kind="ExternalInput")
    xkvT = nc.dram_tensor("xkvT", [D, S], F32R, kind="ExternalInput")
    wqT = nc.dram_tensor("wqT", [D, GO], F32R, kind="ExternalInput")
    wkT = nc.dram_tensor("wkT", [D, GO], F32R, kind="ExternalInput")
    wvT = nc.dram_tensor("wvT", [D, GO], F32R, kind="ExternalInput")
    woT = nc.dram_tensor("woT", [GO, D], F32R, kind="ExternalInput")
    y = nc.dram_tensor("y", [S, D], F32, kind="ExternalOutput")

    Exp = mybir.ActivationFunctionType.Exp

    with tile.TileContext(nc) as tc:
        from contextlib import ExitStack

        with ExitStack() as ctx:
            ctx.enter_context(
                nc.allow_low_precision(reason="fp32r matmul input rounding")
            )
            # ---- persistent SBUF buffers ----
            pers = ctx.enter_context(tc.tile_pool(name="pers", bufs=1))
            qT = [pers.tile([P, S], F32R, tag=f"qT{i}", name=f"qT{i}") for i in range(NM)]
            kT = [pers.tile([P, S], F32R, tag=f"kT{i}", name=f"kT{i}") for i in range(NM)]
            vaug = [pers.tile([P, HG * 65], F32R, tag=f"va{i}", name=f"va{i}") for i in range(NTB)]
            oT = [pers.tile([P, S], F32R, tag=f"oT{i}", name=f"oT{i}") for i in range(NM)]
            consts = ctx.enter_context(tc.tile_pool(name="consts", bufs=1))
            tri_f = consts.tile([P, P], F32)  # tri[k,q] = 1 if q >= k else 0
            make_upper_triangular(nc, tri_f[:], val=1.0, diag=True)
            tri = consts.tile([P, P], F32R)
            nc.vector.tensor_copy(tri[:], tri_f[:])
            ones_f = consts.tile([1, 64], F32)
            nc.vector.memset(ones_f[:], 1.0)
            ones1 = consts.tile([1, 64], F32R)
            nc.vector.tensor_copy(ones1[:], ones_f[:])
            vone = consts.tile([P, HG * 65], F32)
            nc.vector.memset(vone[:], 1.0)
            zero_f = consts.tile([P, P], F32)
            nc.vector.memset(zero_f[:], 0.0)
            zero_r = consts.tile([P, P], F32R)
            nc.vector.tensor_copy(zero_r[:], zero_f[:])
            for i in range(NTB):
                # ones columns survive the V evictions (cols h*65+64)
                nc.vector.tensor_copy(vaug[i][:], vone[:])

            # ---- phase 1a: Q projection ----
            with (
                tc.tile_pool(name="wq", bufs=1) as wq_pool,
                tc.tile_pool(name="xq", bufs=2) as xq_pool,
                tc.tile_pool(name="pj", bufs=2, space="PSUM") as pj_pool,
            ):
                wq_t = [wq_pool.tile([P, GO], F32R, tag=f"w{d}", name=f"wq{d}") for d in range(ND)]
                for d in range(ND):
                    nc.sync.dma_start(wq_t[d][:], wqT[ts(d, P), :])
                for tc_i in range(NCH):
                    xq = [xq_pool.tile([P, TCH], F32R, tag=f"x{d}", name=f"xq{d}_{tc_i}") for d in range(ND)]
                    for d in range(ND):
                        nc.sync.dma_start(xq[d][:], xqT[ts(d, P), ts(tc_i, TCH)])
                    for m in range(NM):
                        ps = pj_pool.tile([P, TCH], F32, tag="pj")
                        for d in range(ND):
                            nc.tensor.matmul(
                                ps[:],
                                wq_t[d][:, ts(m, P)],
                                xq[d][:],
                                start=(d == 0),
                                stop=(d == ND - 1),
                            )
                        nc.vector.tensor_copy(qT[m][:, ts(tc_i, TCH)], ps[:])

            # ---- phase 1b: K and V projections ----
            with (
                tc.tile_pool(name="wkv", bufs=1) as wkv_pool,
                tc.tile_pool(name="xkv", bufs=2) as xkv_pool,
                tc.tile_pool(name="pj2", bufs=2, space="PSUM") as pj2_pool,
            ):
                wk_t = [wkv_pool.tile([P, GO], F32R, tag=f"wk{d}", name=f"wk{d}") for d in range(ND)]
                wv_t = [wkv_pool.tile([P, GO], F32R, tag=f"wv{d}", name=f"wv{d}") for d in range(ND)]
                for d in range(ND):
                    nc.sync.dma_start(wk_t[d][:], wkT[ts(d, P), :])
                    nc.sync.dma_start(wv_t[d][:], wvT[ts(d, P), :])
                for tc_i in range(NCH):
                    xkv = [
                        xkv_pool.tile([P, TCH], F32R, tag=f"x{d}", name=f"xkv{d}_{tc_i}") for d in range(ND)
                    ]
                    for d in range(ND):
                        nc.sync.dma_start(xkv[d][:], xkvT[ts(d, P), ts(tc_i, TCH)])
                    for m in range(NM):
                        ps = pj2_pool.tile([P, TCH], F32, tag="pj")
                        for d in range(ND):
                            nc.tensor.matmul(
                                ps[:],
                                wk_t[d][:, ts(m, P)],
                                xkv[d][:],
                                start=(d == 0),
                                stop=(d == ND - 1),
                            )
                        nc.vector.tensor_copy(kT[m][:, ts(tc_i, TCH)], ps[:])
                    for mt in range(TCH // P):
                        ps = pj2_pool.tile([P, GO], F32, tag="pjv")
                        for d in range(ND):
                            nc.tensor.matmul(
                                ps[:],
                                xkv[d][:, ts(mt, P)],
                                wv_t[d][:],
                                start=(d == 0),
                                stop=(d == ND - 1),
                            )
                        vt = vaug[tc_i * (TCH // P) + mt]
                        nc.vector.tensor_copy(
                            vt[:].rearrange("p (h c) -> p h c", c=65)[:, :, 0:64],
                            ps[:].rearrange("p (h c) -> p h c", c=64),
                        )

            # ---- phase 2: attention (transposed layout) ----
            with (
                tc.tile_pool(name="att", bufs=4) as apool,
                tc.tile_pool(name="attn2", bufs=2) as apool2,
                tc.tile_pool(name="ps_s", bufs=2, space="PSUM") as spool,
                tc.tile_pool(name="ps_o", bufs=1, space="PSUM") as opool,
                tc.tile_pool(name="ps_b", bufs=2, space="PSUM") as bpool,
            ):
                for hp in range(HG // 2):
                    # head pair (2hp, 2hp+1): partitions 0-63 / 64-127 of
                    # tile hp -- their K=64 scores matmuls land in disjoint
                    # PE row groups and run concurrently.
                    ti = hp
                    for qt in range(NQT):
                        psoA = opool.tile([P, 512], F32, tag="oA", name=f"oA{hp}_{qt}")
                        psoB = opool.tile([P, 512], F32, tag="oB", name=f"oB{hp}_{qt}")
                        nkb = 4 * qt + 4
                        for kb in range(nkb):
                            j = kb - 4 * qt
                            ce = max(j, 0) * P
                            c0 = min(ce, 2 * P)
                            pssA = spool.tile([P, 512], F32, tag="sA", name=f"sA{hp}_{qt}_{kb}")
                            pssB = spool.tile([P, 512], F32, tag="sB", name=f"sB{hp}_{qt}_{kb}")
                            for po, pss in ((0, pssA), (64, pssB)):
                                nc.tensor.matmul(
                                    pss[:, c0:],
                                    kT[ti][po : po + 64, ts(kb, P)],
                                    qT[ti][po : po + 64, qt * 512 + c0 : (qt + 1) * 512],
                                    start=True,
                                    stop=True,
                                )
                            pexpA = apool.tile([P, 512], F32R, tag="pA", name=f"pA{hp}_{qt}_{kb}")
                            pexpB = apool.tile([P, 512], F32R, tag="pB", name=f"pB{hp}_{qt}_{kb}")
                            for pss, pexp in ((pssA, pexpA), (pssB, pexpB)):
                                if j == 3:
                                    nc.vector.tensor_copy(
                                        pexp[:, 2 * P : 3 * P], zero_r[:]
                                    )
                                nc.scalar.activation(
                                    pexp[:, ce:], pss[:, ce:], Exp, scale=0.125
                                )
                                if j >= 0:
                                    nc.vector.tensor_mul(
                                        pexp[:, ts(j, P)], pexp[:, ts(j, P)], tri[:]
                                    )
                            for hh, pexp, pso in (
                                (2 * hp, pexpA, psoA),
                                (2 * hp + 1, pexpB, psoB),
                            ):
                                nc.tensor.matmul(
                                    pso[:65, c0:],
                                    vaug[kb][:, hh * 65 : hh * 65 + 65],
                                    pexp[:, c0:],
                                    start=(kb == 0),
                                    stop=(kb == nkb - 1),
                                )
                        for hh, pso in ((2 * hp, psoA), (2 * hp + 1, psoB)):
                            rec = apool2.tile([1, 512], F32R, tag="rec", name=f"rec{hh}_{qt}")
                            nc.vector.reciprocal(rec[:], pso[64:65, :])
                            bc_ps = bpool.tile([64, 512], F32, tag="bc", name=f"bc{hh}_{qt}")
                            nc.tensor.matmul(
                                bc_ps[:],
                                ones1[:],
                                rec[:],
                                start=True,
                                stop=True,
                            )
                            bc_sb = apool2.tile([64, 512], F32, tag="bcs", name=f"bcs{hh}_{qt}")
                            nc.vector.tensor_copy(bc_sb[:], bc_ps[:])
                            row = hh * 64
                            nc.vector.tensor_mul(
                                oT[row // P][row % P : row % P + 64, ts(qt, 512)],
                                pso[0:64, :],
                                bc_sb[:],
                            )

            # ---- phase 3: output projection (partial over this head group) ----
            with (
                tc.tile_pool(name="wo", bufs=1) as wo_pool,
                tc.tile_pool(name="yev", bufs=3) as y_pool,
                tc.tile_pool(name="ps_y", bufs=2, space="PSUM") as ypool,
            ):
                wo_t = [wo_pool.tile([P, D], F32R, tag=f"wo{i}", name=f"wo{i}") for i in range(NM)]
                for i in range(NM):
                    nc.sync.dma_start(wo_t[i][:], woT[ts(i, P), :])
                for mt in range(NTB):
                    for nt in range(D // 512):
                        ps = ypool.tile([P, 512], F32, tag="y")
                        for ob in range(NM):
                            nc.tensor.matmul(
                                ps[:],
                                oT[ob][:, ts(mt, P)],
                                wo_t[ob][:, ts(nt, 512)],
                                start=(ob == 0),
                                stop=(ob == NM - 1),
                            )
                        ysb = y_pool.tile([P, 512], F32, tag="ysb")
                        nc.vector.tensor_copy(ysb[:], ps[:])
                        nc.sync.dma_start(y[ts(mt, P), ts(nt, 512)], ysb[:])
    nc.finalize()
    return nc


_NC_CACHE = {}


def _get_nc():
    if "full" not in _NC_CACHE:
        _NC_CACHE["full"] = build_bass()
    return _NC_CACHE["full"]


def make_in_maps(query, key_value, Wq, Wk, Wv, Wo):
    query = np.asarray(query, dtype=np.float32)
    key_value = np.asarray(key_value, dtype=np.float32)
    Wq, Wk, Wv, Wo = (np.asarray(w, dtype=np.float32) for w in (Wq, Wk, Wv, Wo))
    GO = Wq.shape[0] // 2
    in_maps = []
    for c in range(N_CORES):
        b, g = c // 2, c % 2
        sl = slice(g * GO, (g + 1) * GO)
        in_maps.append(
            {
                "xqT": np.ascontiguousarray(query[b].T),
                "xkvT": np.ascontiguousarray(key_value[b].T),
                "wqT": np.ascontiguousarray(Wq[sl, :].T),
                "wkT": np.ascontiguousarray(Wk[sl, :].T),
                "wvT": np.ascontiguousarray(Wv[sl, :].T),
                "woT": np.ascontiguousarray(Wo[:, sl].T),
            }
        )
    return in_maps


def kernel(query, key_value, Wq, Wk, Wv, Wo):
    from concourse import bass_utils

    nc = _get_nc()
    in_maps = make_in_maps(query, key_value, Wq, Wk, Wv, Wo)
    res = bass_utils.run_bass_kernel_spmd(nc, in_maps, core_ids=list(range(N_CORES)))
    ys = [r["y"] for r in res.results]
    out = np.stack([ys[2 * b] + ys[2 * b + 1] for b in range(B_FULL)])
    return out.astype(np.float32)


# revision 12
# speedup vs baseline: 17.0575x; 1.0176x over previous
"""Causal cross-attention kernel for 8 trn2 NeuronCores.

Sharding: 4-way data-parallel over batch x 2-way tensor-parallel over heads
(8 heads per core).  Each core computes, for its (batch, head-group):
  Q^T = WqT.T-less trick:  Q^T[o,t] = sum_d WqT[d,o] * XqT[d,t]   (o = 512 head dims)
  K^T likewise, V[t,o] natural layout, all via PE matmuls in float32r.
Attention is computed in transposed layout: S^T[k,q] = K^T.T-contraction so the
softmax denominator comes from a ones-column appended to V (no P transpose, no
max-subtraction -- scores are ~N(0,1) so exp never overflows in fp32).
Output projection consumes O^T directly as lhsT; each core emits a full-width
partial y for its batch and the host sums the two head-group partials.

All host-side work (transposes, slicing, pair-sums) is data marshaling; the
device kernel is a single NEFF launch per core.
"""

import sys

sys.path.insert(0, "/opt/trn_rl_repo")

import numpy as np

import concourse.bass as bass
import concourse.tile as tile
from concourse import bacc, mybir
from concourse.bass import ts
from concourse.masks import make_upper_triangular

F32 = mybir.dt.float32
F32R = mybir.dt.float32r
P = 128

# full-problem constants
B_FULL = 4
S_FULL = 2048
D_FULL = 1024
HG_FULL = 8  # heads per core (16 heads / 2-way TP)
N_CORES = 8


def build_bass(S=S_FULL, D=D_FULL, HG=HG_FULL):
    """One-core program; SPMD across 8 cores with different data."""
    GO = HG * 64  # output-feature width of this core's head group
    ND = D // P  # d-blocks (contraction)
    NM = GO // P  # o-tiles of Q/K projections
    NQT = S // 512  # q-tiles (512 wide)
    NTB = S // P  # token blocks of 128
    TCH = 256  # projection t-chunk
    NCH = S // TCH

    nc = bacc.Bacc("TRN2", target_bir_lowering=False, debug=False)
    xqT = nc.dram_tensor("xqT", [D, S], F32R, kind="ExternalInput")
    xkvT = nc.dram_tensor("xkvT", [D, S], F32R, kind="ExternalInput")
    wqT = nc.dram_tensor("wqT", [D, GO], F32R, kind="ExternalInput")
    wkT = nc.dram_tensor("wkT", [D, GO], F32R, kind="ExternalInput")
    wvT = nc.dram_tensor("wvT", [D, GO], F32R, kind="ExternalInput")
    woT = nc.dram_tensor("woT", [GO, D], F32R, kind="ExternalInput")
    y = nc.dram_tensor("y", [S, D], F32, kind="ExternalOutput")

    Exp = mybir.ActivationFunctionType.Exp

    with tile.TileContext(nc) as tc:
        from contextlib import ExitStack

        with ExitStack() as ctx:
            ctx.enter_context(
                nc.allow_low_precision(reason="fp32r matmul input rounding")
            )
            # ---- persistent SBUF buffers ----
            pers = ctx.enter_context(tc.tile_pool(name="pers", bufs=1))
            qT = [pers.tile([P, S], F32R, tag=f"qT{i}", name=f"qT{i}") for i in range(NM)]
            kT = [pers.tile([P, S], F32R, tag=f"kT{i}", name=f"kT{i}") for i in range(NM)]
            vaug = [pers.tile([P, HG * 65], F32R, tag=f"va{i}", name=f"va{i}") for i in range(NTB)]
            consts = ctx.enter_context(tc.tile_pool(name="consts", bufs=1))
            tri_f = consts.tile([P, P], F32)  # tri[k,q] = 1 if q >= k else 0
            make_upper_triangular(nc, tri_f[:], val=1.0, diag=True)
            tri = consts.tile([P, P], F32R)
            nc.vector.tensor_copy(tri[:], tri_f[:])
            ones_f = consts.tile([1, 64], F32)
            nc.vector.memset(ones_f[:], 1.0)
            ones1 = consts.tile([1, 64], F32R)
            nc.vector.tensor_copy(ones1[:], ones_f[:])
            vone = consts.tile([P, HG * 65], F32)
            nc.vector.memset(vone[:], 1.0)
            zero_f = consts.tile([P, P], F32)
            nc.vector.memset(zero_f[:], 0.0)
            zero_r = consts.tile([P, P], F32R)
            nc.vector.tensor_copy(zero_r[:], zero_f[:])
            for i in range(NTB):
                # ones columns survive the V evictions (cols h*65+64)
                nc.vector.tensor_copy(vaug[i][:], vone[:])

            # ---- phase 1: Q/K/V projections (single scope; weights preloaded) ----
            with (
                tc.tile_pool(name="wp", bufs=1) as w_pool,
                tc.tile_pool(name="xp", bufs=2) as x_pool,
                tc.tile_pool(name="pj", bufs=2, space="PSUM") as pj_pool,
            ):
                wq_t = [w_pool.tile([P, GO], F32R, tag=f"wq{d}", name=f"wq{d}") for d in range(ND)]
                wk_t = [w_pool.tile([P, GO], F32R, tag=f"wk{d}", name=f"wk{d}") for d in range(ND)]
                wv_t = [w_pool.tile([P, GO], F32R, tag=f"wv{d}", name=f"wv{d}") for d in range(ND)]
                # DMA issue order: wq, chunk-0 x, then wk/wv -- the first
                # matmul only needs wq[0]+xq[0], so don't queue 4MB of k/v
                # weights ahead of the first activations.
                for d in range(ND):
                    nc.sync.dma_start(wq_t[d][:], wqT[ts(d, P), :])
                x0 = [
                    (
                        x_pool.tile([P, TCH], F32R, tag=f"xq{d}", name=f"xq{d}_0"),
                        x_pool.tile([P, TCH], F32R, tag=f"xk{d}", name=f"xkv{d}_0"),
                    )
                    for d in range(ND)
                ]
                for d in range(ND):
                    nc.sync.dma_start(x0[d][0][:], xqT[ts(d, P), ts(0, TCH)])
                    nc.sync.dma_start(x0[d][1][:], xkvT[ts(d, P), ts(0, TCH)])
                for d in range(ND):
                    nc.sync.dma_start(wk_t[d][:], wkT[ts(d, P), :])
                    nc.sync.dma_start(wv_t[d][:], wvT[ts(d, P), :])
                for tc_i in range(NCH):
                    if tc_i == 0:
                        xq = [t[0] for t in x0]
                        xkv = [t[1] for t in x0]
                    else:
                        xq = [x_pool.tile([P, TCH], F32R, tag=f"xq{d}", name=f"xq{d}_{tc_i}") for d in range(ND)]
                        xkv = [x_pool.tile([P, TCH], F32R, tag=f"xk{d}", name=f"xkv{d}_{tc_i}") for d in range(ND)]
                        for d in range(ND):
                            nc.sync.dma_start(xq[d][:], xqT[ts(d, P), ts(tc_i, TCH)])
                            nc.sync.dma_start(xkv[d][:], xkvT[ts(d, P), ts(tc_i, TCH)])
                    for m in range(NM):
                        ps = pj_pool.tile([P, TCH], F32, tag="pj")
                        for d in range(ND):
                            nc.tensor.matmul(
                                ps[:],
                                wq_t[d][:, ts(m, P)],
                                xq[d][:],
                                start=(d == 0),
                                stop=(d == ND - 1),
                            )
                        nc.vector.tensor_copy(qT[m][:, ts(tc_i, TCH)], ps[:])
                    for m in range(NM):
                        ps = pj_pool.tile([P, TCH], F32, tag="pj")
                        for d in range(ND):
                            nc.tensor.matmul(
                                ps[:],
                                wk_t[d][:, ts(m, P)],
                                xkv[d][:],
                                start=(d == 0),
                                stop=(d == ND - 1),
                            )
                        nc.vector.tensor_copy(kT[m][:, ts(tc_i, TCH)], ps[:])
                    for mt in range(TCH // P):
                        ps = pj_pool.tile([P, GO], F32, tag="pjv")
                        for d in range(ND):
                            nc.tensor.matmul(
                                ps[:],
                                xkv[d][:, ts(mt, P)],
                                wv_t[d][:],
                                start=(d == 0),
                                stop=(d == ND - 1),
                            )
                        vt = vaug[tc_i * (TCH // P) + mt]
                        nc.vector.tensor_copy(
                            vt[:].rearrange("p (h c) -> p h c", c=65)[:, :, 0:64],
                            ps[:].rearrange("p (h c) -> p h c", c=64),
                        )

            # ---- phase 2+3 scope: oT and Wo live across both ----
            tail_ctx = ctx.enter_context(tc.tile_pool(name="tail", bufs=1))
            oT = [tail_ctx.tile([P, S], F32R, tag=f"oT{i}", name=f"oT{i}") for i in range(NM)]
            wo_t = [tail_ctx.tile([P, D], F32R, tag=f"wo{i}", name=f"wo{i}") for i in range(NM)]
            for i in range(NM):
                nc.sync.dma_start(wo_t[i][:], woT[ts(i, P), :])

            # ---- phase 2: attention (transposed layout) ----
            with (
                tc.tile_pool(name="att", bufs=4) as apool,
                tc.tile_pool(name="attn2", bufs=2) as apool2,
                tc.tile_pool(name="ps_s", bufs=2, space="PSUM") as spool,
                tc.tile_pool(name="ps_o", bufs=1, space="PSUM") as opool,
                tc.tile_pool(name="ps_b", bufs=2, space="PSUM") as bpool,
            ):
                for hp in range(HG // 2):
                    # head pair (2hp, 2hp+1): partitions 0-63 / 64-127 of
                    # tile hp -- their K=64 scores matmuls land in disjoint
                    # PE row groups and run concurrently.
                    ti = hp
                    for qt in range(NQT):
                        psoA = opool.tile([P, 512], F32, tag="oA", name=f"oA{hp}_{qt}")
                        psoB = opool.tile([P, 512], F32, tag="oB", name=f"oB{hp}_{qt}")
                        nkb = 4 * qt + 4
                        for kb in range(nkb):
                            j = kb - 4 * qt
                            ce = max(j, 0) * P
                            c0 = min(ce, 2 * P)
                            pssA = spool.tile([P, 512], F32, tag="sA", name=f"sA{hp}_{qt}_{kb}")
                            pssB = spool.tile([P, 512], F32, tag="sB", name=f"sB{hp}_{qt}_{kb}")
                            for po, pss in ((0, pssA), (64, pssB)):
                                nc.tensor.matmul(
                                    pss[:, c0:],
                                    kT[ti][po : po + 64, ts(kb, P)],
                                    qT[ti][po : po + 64, qt * 512 + c0 : (qt + 1) * 512],
                                    start=True,
                                    stop=True,
                                )
                            pexpA = apool.tile([P, 512], F32R, tag="pA", name=f"pA{hp}_{qt}_{kb}")
                            pexpB = apool.tile([P, 512], F32R, tag="pB", name=f"pB{hp}_{qt}_{kb}")
                            for pss, pexp in ((pssA, pexpA), (pssB, pexpB)):
                                if j == 3:
                                    nc.vector.tensor_copy(
                                        pexp[:, 2 * P : 3 * P], zero_r[:]
                                    )
                                nc.scalar.activation(
                                    pexp[:, ce:], pss[:, ce:], Exp, scale=0.125
                                )
                                if j >= 0:
                                    nc.vector.tensor_mul(
                                        pexp[:, ts(j, P)], pexp[:, ts(j, P)], tri[:]
                                    )
                            for hh, pexp, pso in (
                                (2 * hp, pexpA, psoA),
                                (2 * hp + 1, pexpB, psoB),
                            ):
                                nc.tensor.matmul(
                                    pso[:65, c0:],
                                    vaug[kb][:, hh * 65 : hh * 65 + 65],
                                    pexp[:, c0:],
                                    start=(kb == 0),
                                    stop=(kb == nkb - 1),
                                )
                        for hh, pso in ((2 * hp, psoA), (2 * hp + 1, psoB)):
                            rec = apool2.tile([1, 512], F32R, tag="rec", name=f"rec{hh}_{qt}")
                            nc.vector.reciprocal(rec[:], pso[64:65, :])
                            bc_ps = bpool.tile([64, 512], F32, tag="bc", name=f"bc{hh}_{qt}")
                            nc.tensor.matmul(
                                bc_ps[:],
                                ones1[:],
                                rec[:],
                                start=True,
                                stop=True,
                            )
                            bc_sb = apool2.tile([64, 512], F32, tag="bcs", name=f"bcs{hh}_{qt}")
                            nc.vector.tensor_copy(bc_sb[:], bc_ps[:])
                            row = hh * 64
                            nc.vector.tensor_mul(
                                oT[row // P][row % P : row % P + 64, ts(qt, 512)],
                                pso[0:64, :],
                                bc_sb[:],
                            )

            # ---- phase 3: output projection (partial over this head group) ----
            with (
                tc.tile_pool(name="yev", bufs=3) as y_pool,
                tc.tile_pool(name="ps_y", bufs=2, space="PSUM") as ypool,
            ):
                for mt in range(NTB):
                    for nt in range(D // 512):
                        ps = ypool.tile([P, 512], F32, tag="y")
                        for ob in range(NM):
                            nc.tensor.matmul(
                                ps[:],
                                oT[ob][:, ts(mt, P)],
                                wo_t[ob][:, ts(nt, 512)],
                                start=(ob == 0),
                                stop=(ob == NM - 1),
                            )
                        ysb = y_pool.tile([P, 512], F32, tag="ysb")
                        nc.vector.tensor_copy(ysb[:], ps[:])
                        nc.sync.dma_start(y[ts(mt, P), ts(nt, 512)], ysb[:])
    nc.finalize()
    return nc


_NC_CACHE = {}


def _get_nc():
    if "full" not in _NC_CACHE:
        _NC_CACHE["full"] = build_bass()
    return _NC_CACHE["full"]


def make_in_maps(query, key_value, Wq, Wk, Wv, Wo):
    query = np.asarray(query, dtype=np.float32)
    key_value = np.asarray(key_value, dtype=np.float32)
    Wq, Wk, Wv, Wo = (np.asarray(w, dtype=np.float32) for w in (Wq, Wk, Wv, Wo))
    GO = Wq.shape[0] // 2
    in_maps = []
    for c in range(N_CORES):
        b, g = c // 2, c % 2
        sl = slice(g * GO, (g + 1) * GO)
        in_maps.append(
            {
                "xqT": np.ascontiguousarray(query[b].T),
                "xkvT": np.ascontiguousarray(key_value[b].T),
                "wqT": np.ascontiguousarray(Wq[sl, :].T),
                "wkT": np.ascontiguousarray(Wk[sl, :].T),
                "wvT": np.ascontiguousarray(Wv[sl, :].T),
                "woT": np.ascontiguousarray(Wo[:, sl].T),
            }
        )
    return in_maps


def kernel(query, key_value, Wq, Wk, Wv, Wo):
    from concourse import bass_utils

    nc = _get_nc()
    in_maps = make_in_maps(query, key_value, Wq, Wk, Wv, Wo)
    res = bass_utils.run_bass_kernel_spmd(nc, in_maps, core_ids=list(range(N_CORES)))
    ys = [r["y"] for r in res.results]
    out = np.stack([ys[2 * b] + ys[2 * b + 1] for b in range(B_FULL)])
    return out.astype(np.float32)


# revision 17
# speedup vs baseline: 17.9097x; 1.0500x over previous
"""Causal cross-attention kernel for 8 trn2 NeuronCores.

Sharding: 4-way data-parallel over batch x 2-way tensor-parallel over heads
(8 heads per core).  Each core computes, for its (batch, head-group):
  Q^T = WqT.T-less trick:  Q^T[o,t] = sum_d WqT[d,o] * XqT[d,t]   (o = 512 head dims)
  K^T likewise, V[t,o] natural layout, all via PE matmuls in float32r.
Attention is computed in transposed layout: S^T[k,q] = K^T.T-contraction so the
softmax denominator comes from a ones-column appended to V (no P transpose, no
max-subtraction -- scores are ~N(0,1) so exp never overflows in fp32).
Output projection consumes O^T directly as lhsT; each core emits a full-width
partial y for its batch and the host sums the two head-group partials.

All host-side work (transposes, slicing, pair-sums) is data marshaling; the
device kernel is a single NEFF launch per core.
"""

import sys

sys.path.insert(0, "/opt/trn_rl_repo")

import numpy as np

import concourse.bass as bass
import concourse.tile as tile
from concourse import bacc, mybir
from concourse.bass import ts
from concourse.masks import make_upper_triangular

F32 = mybir.dt.float32
F32R = mybir.dt.float32r
P = 128

# full-problem constants
B_FULL = 4
S_FULL = 2048
D_FULL = 1024
HG_FULL = 8  # heads per core (16 heads / 2-way TP)
N_CORES = 8


def build_bass(S=S_FULL, D=D_FULL, HG=HG_FULL):
    """One-core program; SPMD across 8 cores with different data."""
    GO = HG * 64  # output-feature width of this core's head group
    ND = D // P  # d-blocks (contraction)
    NM = GO // P  # o-tiles of Q/K projections
    NQT = S // 512  # q-tiles (512 wide)
    NTB = S // P  # token blocks of 128
    TCH = 256  # projection t-chunk
    NCH = S // TCH

    nc = bacc.Bacc("TRN2", target_bir_lowering=False, debug=False)
    xqT = nc.dram_tensor("xqT", [D, S], F32R, kind="ExternalInput")
    xkvT = nc.dram_tensor("xkvT", [D, S], F32R, kind="ExternalInput")
    wqT = nc.dram_tensor("wqT", [D, GO], F32R, kind="ExternalInput")
    wkT = nc.dram_tensor("wkT", [D, GO], F32R, kind="ExternalInput")
    wvT = nc.dram_tensor("wvT", [D, GO], F32R, kind="ExternalInput")
    woT = nc.dram_tensor("woT", [GO, D], F32R, kind="ExternalInput")
    y = nc.dram_tensor("y", [S, D], F32, kind="ExternalOutput")

    Exp = mybir.ActivationFunctionType.Exp

    with tile.TileContext(nc) as tc:
        from contextlib import ExitStack

        with ExitStack() as ctx:
            ctx.enter_context(
                nc.allow_low_precision(reason="fp32r matmul input rounding")
            )
            # ---- persistent SBUF buffers ----
            pers = ctx.enter_context(tc.tile_pool(name="pers", bufs=1))
            qT = [pers.tile([P, S], F32R, tag=f"qT{i}", name=f"qT{i}") for i in range(NM)]
            kT = [pers.tile([P, S], F32R, tag=f"kT{i}", name=f"kT{i}") for i in range(NM)]
            vaug = [pers.tile([P, HG * 65], F32R, tag=f"va{i}", name=f"va{i}") for i in range(NTB)]
            consts = ctx.enter_context(tc.tile_pool(name="consts", bufs=1))
            tri_f = consts.tile([P, P], F32)  # tri[k,q] = 1 if q >= k else 0
            make_upper_triangular(nc, tri_f[:], val=1.0, diag=True)
            tri = consts.tile([P, P], F32R)
            nc.vector.tensor_copy(tri[:], tri_f[:])
            ones_f = consts.tile([1, 64], F32)
            nc.vector.memset(ones_f[:], 1.0)
            ones1 = consts.tile([1, 64], F32R)
            nc.vector.tensor_copy(ones1[:], ones_f[:])
            vone = consts.tile([P, HG * 65], F32)
            nc.vector.memset(vone[:], 1.0)
            zero_f = consts.tile([P, P], F32)
            nc.vector.memset(zero_f[:], 0.0)
            zero_r = consts.tile([P, P], F32R)
            nc.vector.tensor_copy(zero_r[:], zero_f[:])
            for i in range(NTB):
                # ones columns survive the V evictions (cols h*65+64)
                nc.vector.tensor_copy(vaug[i][:], vone[:])

            # ---- phase 1: Q/K/V projections (single scope; weights preloaded) ----
            with (
                tc.tile_pool(name="wp", bufs=1) as w_pool,
                tc.tile_pool(name="xp", bufs=2) as x_pool,
                tc.tile_pool(name="pj", bufs=2, space="PSUM") as pj_pool,
            ):
                wq_t = [w_pool.tile([P, GO], F32R, tag=f"wq{d}", name=f"wq{d}") for d in range(ND)]
                wk_t = [w_pool.tile([P, GO], F32R, tag=f"wk{d}", name=f"wk{d}") for d in range(ND)]
                wv_t = [w_pool.tile([P, GO], F32R, tag=f"wv{d}", name=f"wv{d}") for d in range(ND)]
                # DMA issue order: wq, chunk-0 x, then wk/wv -- the first
                # matmul only needs wq[0]+xq[0], so don't queue 4MB of k/v
                # weights ahead of the first activations.
                for d in range(ND):
                    nc.sync.dma_start(wq_t[d][:], wqT[ts(d, P), :])
                x0 = [
                    (
                        x_pool.tile([P, TCH], F32R, tag=f"xq{d}", name=f"xq{d}_0"),
                        x_pool.tile([P, TCH], F32R, tag=f"xk{d}", name=f"xkv{d}_0"),
                    )
                    for d in range(ND)
                ]
                for d in range(ND):
                    nc.sync.dma_start(x0[d][0][:], xqT[ts(d, P), ts(0, TCH)])
                    nc.sync.dma_start(x0[d][1][:], xkvT[ts(d, P), ts(0, TCH)])
                for d in range(ND):
                    nc.sync.dma_start(wk_t[d][:], wkT[ts(d, P), :])
                    nc.sync.dma_start(wv_t[d][:], wvT[ts(d, P), :])
                for tc_i in range(NCH):
                    if tc_i == 0:
                        xq = [t[0] for t in x0]
                        xkv = [t[1] for t in x0]
                    else:
                        xq = [x_pool.tile([P, TCH], F32R, tag=f"xq{d}", name=f"xq{d}_{tc_i}") for d in range(ND)]
                        xkv = [x_pool.tile([P, TCH], F32R, tag=f"xk{d}", name=f"xkv{d}_{tc_i}") for d in range(ND)]
                        for d in range(ND):
                            nc.sync.dma_start(xq[d][:], xqT[ts(d, P), ts(tc_i, TCH)])
                            nc.sync.dma_start(xkv[d][:], xkvT[ts(d, P), ts(tc_i, TCH)])
                    for m in range(NM):
                        ps = pj_pool.tile([P, TCH], F32, tag="pj")
                        for d in range(ND):
                            nc.tensor.matmul(
                                ps[:],
                                wq_t[d][:, ts(m, P)],
                                xq[d][:],
                                start=(d == 0),
                                stop=(d == ND - 1),
                            )
                        nc.vector.tensor_copy(qT[m][:, ts(tc_i, TCH)], ps[:])
                    for m in range(NM):
                        ps = pj_pool.tile([P, TCH], F32, tag="pj")
                        for d in range(ND):
                            nc.tensor.matmul(
                                ps[:],
                                wk_t[d][:, ts(m, P)],
                                xkv[d][:],
                                start=(d == 0),
                                stop=(d == ND - 1),
                            )
                        nc.vector.tensor_copy(kT[m][:, ts(tc_i, TCH)], ps[:])
                    for mt in range(TCH // P):
                        ps = pj_pool.tile([P, GO], F32, tag="pjv")
                        for d in range(ND):
                            nc.tensor.matmul(
                                ps[:],
                                xkv[d][:, ts(mt, P)],
                                wv_t[d][:],
                                start=(d == 0),
                                stop=(d == ND - 1),
                            )
                        vt = vaug[tc_i * (TCH // P) + mt]
                        nc.vector.tensor_copy(
                            vt[:].rearrange("p (h c) -> p h c", c=65)[:, :, 0:64],
                            ps[:].rearrange("p (h c) -> p h c", c=64),
                        )

            # ---- phase 2+3 scope: oT and Wo live across both ----
            tail_ctx = ctx.enter_context(tc.tile_pool(name="tail", bufs=1))
            oT = [tail_ctx.tile([P, S], F32R, tag=f"oT{i}", name=f"oT{i}") for i in range(NM)]
            wo_t = [tail_ctx.tile([P, D], F32R, tag=f"wo{i}", name=f"wo{i}") for i in range(NM)]
            for i in range(NM):
                nc.sync.dma_start(wo_t[i][:], woT[ts(i, P), :])

            # ---- phase 2: attention (transposed layout) ----
            with (
                tc.tile_pool(name="att", bufs=4) as apool,
                tc.tile_pool(name="attn2", bufs=2) as apool2,
                tc.tile_pool(name="ps_s", bufs=2, space="PSUM") as spool,
                tc.tile_pool(name="ps_o", bufs=1, space="PSUM") as opool,
                tc.tile_pool(name="ps_b", bufs=2, space="PSUM") as bpool,
            ):
                for hp in range(HG // 2):
                    # head pair (2hp, 2hp+1): partitions 0-63 / 64-127 of
                    # tile hp -- their K=64 scores matmuls land in disjoint
                    # PE row groups and run concurrently.
                    ti = hp
                    for qt in range(NQT):
                        psoA = opool.tile([P, 512], F32, tag="oA", name=f"oA{hp}_{qt}")
                        psoB = opool.tile([P, 512], F32, tag="oB", name=f"oB{hp}_{qt}")
                        nkb = 4 * qt + 4
                        for kb in range(nkb):
                            j = kb - 4 * qt
                            ce = max(j, 0) * P
                            c0 = min(ce, 2 * P)
                            # both heads' scores in one 2-bank PSUM tile so a
                            # single strided activation does both exps (ACT
                            # per-op overhead is the HW attention limiter)
                            pss = spool.tile([P, 1024], F32, tag="s", name=f"s{hp}_{qt}_{kb}")
                            for g, po in ((0, 0), (1, 64)):
                                nc.tensor.matmul(
                                    pss[:, g * 512 + c0 : (g + 1) * 512],
                                    kT[ti][po : po + 64, ts(kb, P)],
                                    qT[ti][po : po + 64, qt * 512 + c0 : (qt + 1) * 512],
                                    start=True,
                                    stop=True,
                                )
                            pexp = apool.tile([P, 1024], F32R, tag="p", name=f"p{hp}_{qt}_{kb}")
                            if j == 3:
                                nc.vector.tensor_copy(pexp[:, 2 * P : 3 * P], zero_r[:])
                                nc.vector.tensor_copy(
                                    pexp[:, 512 + 2 * P : 512 + 3 * P], zero_r[:]
                                )
                            nc.scalar.activation(
                                pexp[:].rearrange("p (g c) -> p g c", g=2)[:, :, ce:],
                                pss[:].rearrange("p (g c) -> p g c", g=2)[:, :, ce:],
                                Exp,
                                scale=0.125,
                            )
                            if j >= 0:
                                nc.vector.tensor_mul(
                                    pexp[:, ts(j, P)], pexp[:, ts(j, P)], tri[:]
                                )
                                nc.vector.tensor_mul(
                                    pexp[:, 512 + j * P : 512 + (j + 1) * P],
                                    pexp[:, 512 + j * P : 512 + (j + 1) * P],
                                    tri[:],
                                )
                            for g, hh, pso in (
                                (0, 2 * hp, psoA),
                                (1, 2 * hp + 1, psoB),
                            ):
                                nc.tensor.matmul(
                                    pso[:65, c0:],
                                    vaug[kb][:, hh * 65 : hh * 65 + 65],
                                    pexp[:, g * 512 + c0 : (g + 1) * 512],
                                    start=(kb == 0),
                                    stop=(kb == nkb - 1),
                                )
                        for hh, pso in ((2 * hp, psoA), (2 * hp + 1, psoB)):
                            rec = apool2.tile([1, 512], F32R, tag="rec", name=f"rec{hh}_{qt}")
                            nc.vector.reciprocal(rec[:], pso[64:65, :])
                            bc_ps = bpool.tile([64, 512], F32, tag="bc", name=f"bc{hh}_{qt}")
                            nc.tensor.matmul(
                                bc_ps[:],
                                ones1[:],
                                rec[:],
                                start=True,
                                stop=True,
                            )
                            bc_sb = apool2.tile([64, 512], F32, tag="bcs", name=f"bcs{hh}_{qt}")
                            nc.vector.tensor_copy(bc_sb[:], bc_ps[:])
                            row = hh * 64
                            nc.vector.tensor_mul(
                                oT[row // P][row % P : row % P + 64, ts(qt, 512)],
                                pso[0:64, :],
                                bc_sb[:],
                            )

            # ---- phase 3: output projection (partial over this head group) ----
            with (
                tc.tile_pool(name="yev", bufs=3) as y_pool,
                tc.tile_pool(name="ps_y", bufs=2, space="PSUM") as ypool,
            ):
                for mt in range(NTB):
                    for nt in range(D // 512):
                        ps = ypool.tile([P, 512], F32, tag="y")
                        for ob in range(NM):
                            nc.tensor.matmul(
                                ps[:],
                                oT[ob][:, ts(mt, P)],
                                wo_t[ob][:, ts(nt, 512)],
                                start=(ob == 0),
                                stop=(ob == NM - 1),
                            )
                        ysb = y_pool.tile([P, 512], F32, tag="ysb")
                        nc.vector.tensor_copy(ysb[:], ps[:])
                        nc.sync.dma_start(y[ts(mt, P), ts(nt, 512)], ysb[:])
    nc.finalize()
    return nc


_NC_CACHE = {}


def _get_nc():
    if "full" not in _NC_CACHE:
        _NC_CACHE["full"] = build_bass()
    return _NC_CACHE["full"]


def make_in_maps(query, key_value, Wq, Wk, Wv, Wo):
    query = np.asarray(query, dtype=np.float32)
    key_value = np.asarray(key_value, dtype=np.float32)
    Wq, Wk, Wv, Wo = (np.asarray(w, dtype=np.float32) for w in (Wq, Wk, Wv, Wo))
    GO = Wq.shape[0] // 2
    in_maps = []
    for c in range(N_CORES):
        b, g = c // 2, c % 2
        sl = slice(g * GO, (g + 1) * GO)
        in_maps.append(
            {
                "xqT": np.ascontiguousarray(query[b].T),
                "xkvT": np.ascontiguousarray(key_value[b].T),
                "wqT": np.ascontiguousarray(Wq[sl, :].T),
                "wkT": np.ascontiguousarray(Wk[sl, :].T),
                "wvT": np.ascontiguousarray(Wv[sl, :].T),
                "woT": np.ascontiguousarray(Wo[:, sl].T),
            }
        )
    return in_maps


def kernel(query, key_value, Wq, Wk, Wv, Wo):
    from concourse import bass_utils

    nc = _get_nc()
    in_maps = make_in_maps(query, key_value, Wq, Wk, Wv, Wo)
    res = bass_utils.run_bass_kernel_spmd(nc, in_maps, core_ids=list(range(N_CORES)))
    ys = [r["y"] for r in res.results]
    out = np.stack([ys[2 * b] + ys[2 * b + 1] for b in range(B_FULL)])
    return out.astype(np.float32)
